# revision 1
# baseline (speedup 1.0000x reference)
"""GQA attention (llama3-style RoPE, causal) on 8 trn2 NeuronCores.

Sharding: tensor-parallel over KV-head groups. Core i owns kv-head i and its
4 query heads: wq[:, i*512:(i+1)*512], wk/wv[:, i*128:(i+1)*128], and the
matching row-slice wo[i*512:(i+1)*512, :]. After the partial o_proj, a
ReduceScatter(add) over the sequence dim leaves core i with output rows
[i*256, (i+1)*256); the host concatenates the shards.

On-device layout (per core): everything is computed transposed-by-design so
no PE transposes are needed in the hot path:
  qT/kT  [d=128, T]   = wq/wk-tile.T @ xT          (lhsT=weight, rhs=xT)
  vT     [d, T]       -> PE-transposed to v [T, d] (16 small transposes)
  sT     [tk, tq]     = k @ qT                     (lhsT=kT-tile, rhs=qT)
  pT     = exp(sT/sqrt(d)) * causal-mask
  l(row) [1, tq]      = ones.T @ pT                (M=1 matmul, PSUM-accum)
  oT     [d, tq]      = v.T @ pT                   (PSUM-accum over tk)
  o_norm = oT * (1/l) partition-broadcast
  partial[t, :]       = out_heads @ wo_i           (lhsT=oT-tile, rhs=wo)
All matmuls run as float32r (1 cycle/row vs 4 for fp32; ~1e-4 rel err).
"""

import numpy as np

H, KV, HD, HID = 32, 8, 128, 4096
T = 2048
N_CORES = 8
QH = H // KV            # 4 query heads per core
DQ = QH * HD            # 512
KT = HID // 128         # 32 contraction tiles for projections
TN = T // 128           # 16 sequence tiles
G = 4                   # tq groups of 512
GW = T // G             # 512
TS = T // N_CORES       # 256 output rows per core after ReduceScatter

THETA, FACTOR, HI_FF, LO_FF, ORIG_MAX = 500000.0, 8.0, 4.0, 1.0, 8192

_CACHE = {}


def _rope_tables():
    inv = 1.0 / (THETA ** (np.arange(0, HD, 2, dtype=np.float64) / HD))
    wavelen = 2.0 * np.pi / inv
    low_wl = ORIG_MAX / LO_FF
    high_wl = ORIG_MAX / HI_FF
    smooth = (ORIG_MAX / wavelen - LO_FF) / (HI_FF - LO_FF)
    scaled = np.where(wavelen > low_wl, inv / FACTOR, inv)
    mid = (wavelen <= low_wl) & (wavelen >= high_wl)
    scaled = np.where(mid, (1 - smooth) * inv / FACTOR + smooth * inv, scaled)
    inv32 = scaled.astype(np.float32)
    pos = np.arange(T, dtype=np.float32)
    freqs = pos[:, None] * inv32[None, :]          # [T, 64]
    emb = np.concatenate([freqs, freqs], axis=-1)  # [T, 128]
    cosT = np.ascontiguousarray(np.cos(emb).T)     # [128, T]
    sinT = np.ascontiguousarray(np.sin(emb).T)
    return cosT, sinT


def _causal_masks():
    # pT tile is [tk(part) 128, tq(free) 512]; within a tq-group the diagonal
    # tile sits at block v (=tk_tile - 4*g). keep where tq >= tk.
    tri = np.triu(np.ones((128, 128), dtype=np.float32))
    masks = np.zeros((4, 128, 512), dtype=np.float32)
    for v in range(4):
        for c in range(4):
            if c > v:
                masks[v, :, c * 128:(c + 1) * 128] = 1.0
            elif c == v:
                masks[v, :, c * 128:(c + 1) * 128] = tri
    return masks


def _build_program():
    import concourse.bacc as bacc
    import concourse.mybir as mybir
    from concourse.tile import TileContext

    f32 = mybir.dt.float32
    f32r = mybir.dt.float32r
    EXPF = mybir.ActivationFunctionType.Exp

    nc = bacc.Bacc("TRN2", target_bir_lowering=False, debug=False,
                   num_devices=N_CORES)

    xT = nc.dram_tensor("xT", [HID, T], f32, kind="ExternalInput")
    wqd = nc.dram_tensor("wq", [HID, DQ], f32, kind="ExternalInput")
    wkd = nc.dram_tensor("wk", [HID, HD], f32, kind="ExternalInput")
    wvd = nc.dram_tensor("wv", [HID, HD], f32, kind="ExternalInput")
    wod = nc.dram_tensor("wo", [DQ, HID], f32, kind="ExternalInput")
    cosd = nc.dram_tensor("cosT", [HD, T], f32, kind="ExternalInput")
    sind = nc.dram_tensor("sinT", [HD, T], f32, kind="ExternalInput")
    maskd = nc.dram_tensor("masks", [4, HD, GW], f32, kind="ExternalInput")
    identd = nc.dram_tensor("ident", [128, 128], f32, kind="ExternalInput")
    onesd = nc.dram_tensor("ones", [128, 1], f32, kind="ExternalInput")
    outd = nc.dram_tensor("out", [TS, HID], f32, kind="ExternalOutput")

    partials = [nc.dram_tensor(f"partial{g}", [GW, HID], f32) for g in range(G)]
    rs_outs = [nc.dram_tensor(f"rs_out{g}", [GW // N_CORES, HID], f32)
               for g in range(G)]

    def r(ap):
        return ap.bitcast(f32r)

    with TileContext(nc) as tc:
        with (
            tc.tile_pool(name="const", bufs=1) as cpool,
            tc.tile_pool(name="qkv", bufs=1) as qkv,
            tc.tile_pool(name="obuf", bufs=4) as obp,
        ):
            # ---- constants (DMAs deferred into g==0 to unblock first matmuls) ----
            cos = cpool.tile([HD, T], f32, tag="cos")
            sin = cpool.tile([HD, T], f32, tag="sin")
            ident = cpool.tile([128, 128], f32r, tag="ident")
            ones = cpool.tile([128, 1], f32r, tag="ones")
            # persistent activations
            qT = [qkv.tile([128, T], f32r, tag=f"qT{h}", name=f"qT{h}") for h in range(QH)]
            kTt = qkv.tile([128, T], f32r, tag="kT")
            vsb = qkv.tile([128, T], f32r, tag="vsb")  # v tiles side by side

            # ---- phase B: projections + RoPE (+ v transpose) ----
            with (
                tc.tile_pool(name="wq", bufs=1) as wqp,
                tc.tile_pool(name="stream", bufs=3) as stp,
                tc.tile_pool(name="tmp", bufs=2) as tmp,
                tc.tile_pool(name="ppsum", bufs=1, space="PSUM") as pps,
                tc.tile_pool(name="trpsum", bufs=2, space="PSUM") as trp,
            ):
                wq_t = [None] * KT

                for g in range(G):
                    gs = slice(g * GW, (g + 1) * GW)
                    qps = [pps.tile([128, GW], f32, tag=f"qp{h}", name=f"qp{h}_{g}") for h in range(QH)]
                    kps = pps.tile([128, GW], f32, tag="kp")
                    vps = pps.tile([128, GW], f32, tag="vp")
                    for k in range(KT):
                        xt = stp.tile([128, GW], f32r, tag="xt")
                        nc.sync.dma_start(xt[:], r(xT[k * 128:(k + 1) * 128, gs]))
                        if wq_t[k] is None:
                            wt = wqp.tile([128, DQ], f32r, tag=f"wq{k}", name=f"wq{k}")
                            nc.sync.dma_start(wt[:], r(wqd[k * 128:(k + 1) * 128, :]))
                            wq_t[k] = wt
                        wkt = stp.tile([128, HD], f32r, tag="wkt")
                        nc.sync.dma_start(wkt[:], r(wkd[k * 128:(k + 1) * 128, :]))
                        wvt = stp.tile([128, HD], f32r, tag="wvt")
                        nc.sync.dma_start(wvt[:], r(wvd[k * 128:(k + 1) * 128, :]))
                        st = (k == 0)
                        sp = (k == KT - 1)
                        for h in range(QH):
                            nc.tensor.matmul(qps[h][:], wq_t[k][:, h * 128:(h + 1) * 128],
                                             xt[:], start=st, stop=sp)
                        nc.tensor.matmul(kps[:], wkt[:], xt[:], start=st, stop=sp)
                        nc.tensor.matmul(vps[:], wvt[:], xt[:], start=st, stop=sp)

                    if g == 0:
                        nc.sync.dma_start(cos[:], cosd[:])
                        nc.sync.dma_start(sin[:], sind[:])
                        nc.sync.dma_start(ident[:], r(identd[:]))
                        nc.sync.dma_start(ones[:], r(onesd[:]))

                    # RoPE drain for q heads and k; v transpose
                    for h in range(QH + 1):
                        src = qps[h] if h < QH else kps
                        dst = qT[h] if h < QH else kTt
                        t1 = tmp.tile([128, GW], f32, tag="t1")
                        nc.vector.tensor_mul(t1[:], src[:], cos[:, gs])
                        rot = tmp.tile([128, GW], f32, tag="rot")
                        nc.scalar.mul(rot[0:64, :], src[64:128, :], -1.0)
                        nc.scalar.copy(rot[64:128, :], src[0:64, :])
                        rot2 = tmp.tile([128, GW], f32, tag="rot2")
                        nc.vector.tensor_mul(rot2[:], rot[:], sin[:, gs])
                        nc.vector.tensor_add(dst[:, gs].bitcast(f32r), t1[:], rot2[:])

                    vTt = tmp.tile([128, GW], f32r, tag="vT")
                    nc.vector.tensor_copy(vTt[:], vps[:])
                    for ts in range(4):
                        tp = trp.tile([128, 128], f32r, tag="trp")
                        nc.tensor.transpose(tp[:], vTt[:, ts * 128:(ts + 1) * 128], ident[:])
                        nc.vector.tensor_copy(
                            vsb[:, (4 * g + ts) * 128:(4 * g + ts + 1) * 128].bitcast(f32r),
                            tp[:])

            # ---- phases C+D+E interleaved: attention (g-outer) -> o_proj for
            # the finished tq block -> chunked ReduceScatter (overlaps compute) ----
            scale = float(1.0 / np.sqrt(HD))
            with (
                tc.tile_pool(name="mask", bufs=1) as mpool,
                tc.tile_pool(name="oT", bufs=1) as otp,
                tc.tile_pool(name="wor", bufs=1) as worp,
                tc.tile_pool(name="wos", bufs=2) as wop,
                tc.tile_pool(name="pt", bufs=4) as ptp,
                tc.tile_pool(name="norm", bufs=2) as nrm,
                tc.tile_pool(name="apsum", bufs=2, space="PSUM") as aps,
                tc.tile_pool(name="apsum1", bufs=2, space="PSUM") as aps1,
                tc.tile_pool(name="opsum", bufs=2, space="PSUM") as opsum,
            ):
                mtiles = []
                for v in range(4):
                    mt = mpool.tile([HD, GW], f32r, tag=f"mask{v}", name=f"mask{v}")
                    nc.sync.dma_start(mt[:], r(maskd[v]))
                    mtiles.append(mt)
                oT = [otp.tile([128, T], f32r, tag=f"oT{h}", name=f"oT{h}") for h in range(QH)]
                # wo: n-chunks 0..3 resident, 4..7 streamed per g
                wo_res = []
                for f in range(QH):
                    wc = worp.tile([128, 2048], f32r, tag=f"wor{f}", name=f"wor{f}")
                    nc.sync.dma_start(wc[:], r(wod[f * 128:(f + 1) * 128, 0:2048]))
                    wo_res.append(wc)

                for g in range(G):
                    gs = slice(g * GW, (g + 1) * GW)
                    nj = 4 * g + 4
                    for h in range(QH):
                        ops_ = aps1.tile([128, GW], f32, tag="op")
                        lps = aps1.tile([1, GW], f32, tag="lp")
                        for j in range(nj):
                            sps = aps.tile([128, GW], f32, tag="sp")
                            nc.tensor.matmul(sps[:], kTt[:, j * 128:(j + 1) * 128],
                                             qT[h][:, gs], start=True, stop=True)
                            pt = ptp.tile([128, GW], f32r, tag="pt")
                            nc.scalar.activation(pt[:], sps[:], EXPF, scale=scale)
                            if j >= 4 * g:
                                nc.vector.tensor_mul(pt[:], pt[:], mtiles[j - 4 * g][:])
                            nc.tensor.matmul(lps[:], ones[:], pt[:],
                                             start=(j == 0), stop=(j == nj - 1))
                            nc.tensor.matmul(ops_[:], vsb[:, j * 128:(j + 1) * 128],
                                             pt[:], start=(j == 0), stop=(j == nj - 1))
                        ls = nrm.tile([1, GW], f32, tag="ls")
                        nc.vector.reciprocal(ls[:], lps[:])
                        lb = nrm.tile([128, GW], f32, tag="lb")
                        nc.gpsimd.partition_broadcast(lb[:], ls[:])
                        nc.vector.tensor_mul(oT[h][:, gs].bitcast(f32r), ops_[:], lb[:])

                    # o_proj for this tq block (t tiles 4g..4g+3)
                    for n in range(HID // 512):
                        if n < 4:
                            wo_c = [wo_res[f][:, n * 512:(n + 1) * 512] for f in range(QH)]
                        else:
                            wo_c = []
                            for f in range(QH):
                                wc = wop.tile([128, 512], f32r, tag=f"woc{f}",
                                              name=f"woc{f}_{g}_{n}")
                                nc.sync.dma_start(
                                    wc[:],
                                    r(wod[f * 128:(f + 1) * 128, n * 512:(n + 1) * 512]))
                                wo_c.append(wc[:])
                        for t in range(4 * g, 4 * g + 4):
                            ops_ = opsum.tile([128, 512], f32, tag="oproj")
                            for f in range(QH):
                                nc.tensor.matmul(ops_[:], oT[f][:, t * 128:(t + 1) * 128],
                                                 wo_c[f],
                                                 start=(f == 0), stop=(f == QH - 1))
                            ob = obp.tile([128, 512], f32, tag="ob")
                            nc.vector.tensor_copy(ob[:], ops_[:])
                            nc.sync.dma_start(
                                partials[g][(t - 4 * g) * 128:(t - 4 * g + 1) * 128,
                                            n * 512:(n + 1) * 512],
                                ob[:])

                    # chunked ReduceScatter for rows [g*512, (g+1)*512):
                    # core i receives rows g*512 + i*64 .. +64 -> outd[g*64:(g+1)*64]
                    nc.gpsimd.collective_compute(
                        "ReduceScatter", mybir.AluOpType.add,
                        replica_groups=[list(range(N_CORES))],
                        ins=[partials[g][:]], outs=[rs_outs[g][:]],
                    )
                    nc.sync.dma_start(
                        outd[g * (GW // N_CORES):(g + 1) * (GW // N_CORES), :],
                        rs_outs[g][:])

    nc.compile()
    return nc


def _get_program():
    if "nc" not in _CACHE:
        _CACHE["nc"] = _build_program()
    return _CACHE["nc"]


def kernel(x, wq, wk, wv, wo):
    from concourse.bass_utils import run_bass_kernel_spmd

    nc = _get_program()

    x2 = np.asarray(x, dtype=np.float32).reshape(T, HID)
    xT = np.ascontiguousarray(x2.T)
    cosT, sinT = _rope_tables()
    masks = _causal_masks()
    ident = np.eye(128, dtype=np.float32)
    ones = np.ones((128, 1), dtype=np.float32)

    wq = np.asarray(wq, dtype=np.float32)
    wk = np.asarray(wk, dtype=np.float32)
    wv = np.asarray(wv, dtype=np.float32)
    wo = np.asarray(wo, dtype=np.float32)

    in_maps = []
    for i in range(N_CORES):
        in_maps.append({
            "xT": xT,
            "wq": np.ascontiguousarray(wq[:, i * DQ:(i + 1) * DQ]),
            "wk": np.ascontiguousarray(wk[:, i * HD:(i + 1) * HD]),
            "wv": np.ascontiguousarray(wv[:, i * HD:(i + 1) * HD]),
            "wo": np.ascontiguousarray(wo[i * DQ:(i + 1) * DQ, :]),
            "cosT": cosT,
            "sinT": sinT,
            "masks": masks,
            "ident": ident,
            "ones": ones,
        })

    _CACHE["last_in_maps"] = in_maps
    res = run_bass_kernel_spmd(nc, in_maps, list(range(N_CORES)))
    _CACHE["last_result"] = res
    # chunked RS layout: core i's rows [g*64:(g+1)*64] are global rows
    # g*512 + i*64 .. g*512 + (i+1)*64
    W8 = GW // N_CORES
    out = np.empty((T, HID), dtype=np.float32)
    for i in range(N_CORES):
        oi = res.results[i]["out"]
        for g in range(G):
            out[g * GW + i * W8:g * GW + (i + 1) * W8] = oi[g * W8:(g + 1) * W8]
    return out.reshape(1, T, HID)



# revision 3
# speedup vs baseline: 1.3467x; 1.3467x over previous
"""GQA attention (llama3-style RoPE, causal) on 8 trn2 NeuronCores.

Sharding: tensor-parallel over KV-head groups for QKV+attention; the o_proj
is COLUMN-parallel. Core i owns kv-head i and its 4 query heads:
wq[:, i*512:(i+1)*512], wk/wv[:, i*128:(i+1)*128], plus the COLUMN slice
wo[:, i*512:(i+1)*512]. After attention, a chunked AllGather (bf16, one per
tq-block of 512) distributes every core's head-transposed attention output
[512, 512] -> gathered [4096, 512]; each core then computes its 512 output
columns for ALL rows (out[t, i*512:(i+1)*512]) with no further collective.
The host concatenates column blocks.

Dataflow per core (everything transposed-by-design, no PE transposes):
  proj:  qT/kT [d=128, T] = w-tile.T @ xT-tile, bf16 weights/x, fp32 PSUM,
         3 PSUM banks (two half-sweeps per tq-block; x streamed twice)
  RoPE:  ACT does the half-swap, DVE the cos/sin muls; outputs bf16
  vT:    staged to DRAM bf16, read back via DMA-transpose -> v [t, d]
  sT  [tk, tq] = k-tile @ qT      (bf16)
  pT  = exp(sT/sqrt(d)) (ACT, bf16 out) * causal-mask (DVE)
  l   = ones.T @ pT               (M=1 matmul, fp32 PSUM accum over tk)
  oT  [d, tq] = v.T @ pT          (fp32 PSUM accum over tk)
  oT_norm = oT * (1/l)            (partition-broadcast), cast bf16 -> AG
  oproj: out[t, 0:512] += agT-tile.T @ wo-tile  (bf16, fp32 PSUM, 32 k-tiles)
All matmuls bf16 (1 cycle/row + fast weight load); PSUM accumulation fp32.
"""

import numpy as np

H, KV, HD, HID = 32, 8, 128, 4096
T = 2048
N_CORES = 8
QH = H // KV            # 4 query heads per core
DQ = QH * HD            # 512
KT = HID // 128         # 32 contraction tiles for projections
TN = T // 128           # 16 sequence tiles
G = 4                   # tq groups of 512
GW = T // G             # 512
NO = HID // N_CORES     # 512 output columns per core

THETA, FACTOR, HI_FF, LO_FF, ORIG_MAX = 500000.0, 8.0, 4.0, 1.0, 8192

_CACHE = {}


def _rope_tables():
    inv = 1.0 / (THETA ** (np.arange(0, HD, 2, dtype=np.float64) / HD))
    wavelen = 2.0 * np.pi / inv
    low_wl = ORIG_MAX / LO_FF
    high_wl = ORIG_MAX / HI_FF
    smooth = (ORIG_MAX / wavelen - LO_FF) / (HI_FF - LO_FF)
    scaled = np.where(wavelen > low_wl, inv / FACTOR, inv)
    mid = (wavelen <= low_wl) & (wavelen >= high_wl)
    scaled = np.where(mid, (1 - smooth) * inv / FACTOR + smooth * inv, scaled)
    inv32 = scaled.astype(np.float32)
    pos = np.arange(T, dtype=np.float32)
    freqs = pos[:, None] * inv32[None, :]          # [T, 64]
    emb = np.concatenate([freqs, freqs], axis=-1)  # [T, 128]
    cosT = np.ascontiguousarray(np.cos(emb).T)     # [128, T]
    sinT = np.ascontiguousarray(np.sin(emb).T)
    return cosT, sinT


def _causal_masks():
    # pT tile is [tk(part) 128, tq(free) 512]; within a tq-group the diagonal
    # tile sits at block v (=tk_tile - 4*g). keep where tq >= tk.
    tri = np.triu(np.ones((128, 128), dtype=np.float32))
    masks = np.zeros((4, 128, 512), dtype=np.float32)
    for v in range(4):
        for c in range(4):
            if c > v:
                masks[v, :, c * 128:(c + 1) * 128] = 1.0
            elif c == v:
                masks[v, :, c * 128:(c + 1) * 128] = tri
    return masks


def _build_program():
    import concourse.bacc as bacc
    import concourse.mybir as mybir
    from concourse.tile import TileContext

    f32 = mybir.dt.float32
    bf16 = mybir.dt.bfloat16
    EXPF = mybir.ActivationFunctionType.Exp

    nc = bacc.Bacc("TRN2", target_bir_lowering=False, debug=False,
                   num_devices=N_CORES)

    xT = nc.dram_tensor("xT", [HID, T], bf16, kind="ExternalInput")
    wqd = nc.dram_tensor("wq", [HID, DQ], bf16, kind="ExternalInput")
    wkd = nc.dram_tensor("wk", [HID, HD], bf16, kind="ExternalInput")
    wvd = nc.dram_tensor("wv", [HID, HD], bf16, kind="ExternalInput")
    wod = nc.dram_tensor("wo", [HID, NO], bf16, kind="ExternalInput")
    cosd = nc.dram_tensor("cosT", [HD, T], f32, kind="ExternalInput")
    sind = nc.dram_tensor("sinT", [HD, T], f32, kind="ExternalInput")
    maskd = nc.dram_tensor("masks", [4, HD, GW], bf16, kind="ExternalInput")
    onesd = nc.dram_tensor("ones", [128, 1], bf16, kind="ExternalInput")
    outd = nc.dram_tensor("out", [T, NO], f32, kind="ExternalOutput")

    vstage = nc.dram_tensor("vstage", [HD, T], bf16)
    ag_ins = [nc.dram_tensor(f"ag_in{g}", [DQ, GW], bf16) for g in range(G)]
    ag_outs = [nc.dram_tensor(f"ag_out{g}", [HID, GW], bf16,
                              addr_space="Shared") for g in range(G)]

    scale = float(1.0 / np.sqrt(HD))

    with TileContext(nc) as tc:
        with (
            tc.tile_pool(name="const", bufs=1) as cpool,
            tc.tile_pool(name="wres", bufs=1) as wres,
            tc.tile_pool(name="stream", bufs=8) as stp,
            tc.tile_pool(name="qkv", bufs=2) as qkv,
            tc.tile_pool(name="kvres", bufs=1) as kvres,
            tc.tile_pool(name="rope", bufs=2) as rtp,
            tc.tile_pool(name="pt", bufs=4) as ptp,
            tc.tile_pool(name="norm", bufs=2) as nrm,
            tc.tile_pool(name="agbuf", bufs=1) as agp,
            tc.tile_pool(name="obuf", bufs=2) as obp,
            tc.tile_pool(name="ppsum", bufs=3, space="PSUM") as pps,
            tc.tile_pool(name="spsum", bufs=2, space="PSUM") as sps_pool,
            tc.tile_pool(name="opsum", bufs=1, space="PSUM") as ops_pool,
            tc.tile_pool(name="lpsum", bufs=1, space="PSUM") as lps_pool,
            tc.tile_pool(name="ojpsum", bufs=1, space="PSUM") as ojp,
        ):
            # resident tiles, DMAs emitted lazily on first use
            cos = cpool.tile([HD, T], f32, tag="cos")
            sin = cpool.tile([HD, T], f32, tag="sin")
            ones = cpool.tile([128, 1], bf16, tag="ones")
            mtiles = [cpool.tile([HD, GW], bf16, tag=f"mask{v}", name=f"mask{v}")
                      for v in range(4)]
            wq_t = [wres.tile([128, DQ], bf16, tag=f"wq{k}", name=f"wq{k}") for k in range(KT)]
            wk_t = [wres.tile([128, HD], bf16, tag=f"wk{k}", name=f"wk{k}") for k in range(KT)]
            wv_t = [wres.tile([128, HD], bf16, tag=f"wv{k}", name=f"wv{k}") for k in range(KT)]
            wo_t = [wres.tile([128, NO], bf16, tag=f"wo{k}", name=f"wo{k}") for k in range(KT)]
            kT_t = [kvres.tile([128, GW], bf16, tag=f"kT{g}", name=f"kT{g}") for g in range(G)]
            v_t = [kvres.tile([128, 128], bf16, tag=f"v{j}", name=f"v{j}") for j in range(TN)]

            def rope_drain(dst, src, gs):
                # dst[bf16] = src*cos + rotate_half(src)*sin ; src is fp32 PSUM
                rot = rtp.tile([128, GW], f32, tag="rot")
                nc.scalar.mul(rot[0:64, :], src[64:128, :], -1.0)
                nc.scalar.copy(rot[64:128, :], src[0:64, :])
                t1 = rtp.tile([128, GW], f32, tag="t1")
                nc.vector.tensor_mul(t1[:], src[:], cos[:, gs])
                nc.vector.tensor_mul(rot[:], rot[:], sin[:, gs])
                nc.vector.tensor_add(dst, t1[:], rot[:])

            def oproj(g):
                # out[t, :NO] for t-block g from gathered attnT (all heads)
                gs = slice(g * GW, (g + 1) * GW)
                ag_t = []
                for f in range(KT):
                    at = agp.tile([128, GW], bf16, tag=f"ag{f}", name=f"ag{f}_{g}")
                    nc.sync.dma_start(at[:], ag_outs[g][f * 128:(f + 1) * 128, :])
                    ag_t.append(at)
                for t in range(4):
                    opj = ojp.tile([128, NO], f32, tag="oj")
                    for f in range(KT):
                        nc.tensor.matmul(opj[:], ag_t[f][:, t * 128:(t + 1) * 128],
                                         wo_t[f][:], start=(f == 0),
                                         stop=(f == KT - 1))
                    ob = obp.tile([128, NO], f32, tag="ob")
                    nc.vector.tensor_copy(ob[:], opj[:])
                    nc.sync.dma_start(outd[g * GW + t * 128:g * GW + (t + 1) * 128, :],
                                      ob[:])

            qT_cur = [None] * QH   # per-g rotating qT tiles
            for g in range(G):
                gs = slice(g * GW, (g + 1) * GW)
                # ---- projections: two half-sweeps over 3 PSUM banks ----
                for half in range(2):
                    h0, h1 = (0, 1) if half == 0 else (2, 3)
                    pp = [pps.tile([128, GW], f32, tag="pp", name=f"pp{g}_{half}_{_i}") for _i in range(3)]
                    for k in range(KT):
                        xt = stp.tile([128, GW], bf16, tag="xt")
                        nc.sync.dma_start(xt[:], xT[k * 128:(k + 1) * 128, gs])
                        if g == 0 and half == 0:
                            nc.sync.dma_start(wq_t[k][:], wqd[k * 128:(k + 1) * 128, :])
                            nc.sync.dma_start(wk_t[k][:], wkd[k * 128:(k + 1) * 128, :])
                        if g == 0 and half == 1 and k == 0:
                            for kk in range(KT):
                                nc.sync.dma_start(wv_t[kk][:],
                                                  wvd[kk * 128:(kk + 1) * 128, :])
                        st, sp = (k == 0), (k == KT - 1)
                        nc.tensor.matmul(pp[0][:], wq_t[k][:, h0 * 128:(h0 + 1) * 128],
                                         xt[:], start=st, stop=sp)
                        nc.tensor.matmul(pp[1][:], wq_t[k][:, h1 * 128:(h1 + 1) * 128],
                                         xt[:], start=st, stop=sp)
                        wkv = wk_t[k] if half == 0 else wv_t[k]
                        nc.tensor.matmul(pp[2][:], wkv[:], xt[:], start=st, stop=sp)
                    if g == 0 and half == 0:
                        nc.sync.dma_start(cos[:], cosd[:])
                        nc.sync.dma_start(sin[:], sind[:])
                        nc.sync.dma_start(ones[:], onesd[:])
                        for v in range(4):
                            nc.sync.dma_start(mtiles[v][:], maskd[v])
                    # drains
                    for i, h in enumerate((h0, h1)):
                        qt = qkv.tile([128, GW], bf16, tag=f"qT{h}", name=f"qT{h}_{g}")
                        rope_drain(qt[:], pp[i], gs)
                        qT_cur[h] = qt
                    if half == 0:
                        rope_drain(kT_t[g][:], pp[2], gs)
                    else:
                        vt = qkv.tile([128, GW], bf16, tag="vT")
                        nc.vector.tensor_copy(vt[:], pp[2][:])
                        nc.sync.dma_start(vstage[:, gs], vt[:])
                        for ts in range(4):
                            j = 4 * g + ts
                            nc.sync.dma_start_transpose(
                                v_t[j][:], vstage[:, j * 128:(j + 1) * 128])

                # ---- attention for tq-block g ----
                nj = 4 * g + 4
                for h in range(QH):
                    ops_ = ops_pool.tile([128, GW], f32, tag="op")
                    lps = lps_pool.tile([1, GW], f32, tag="lp")
                    for j in range(nj):
                        sps = sps_pool.tile([128, GW], f32, tag="sp")
                        nc.tensor.matmul(sps[:], kT_t[j // 4][:, (j % 4) * 128:(j % 4 + 1) * 128],
                                         qT_cur[h][:], start=True, stop=True)
                        pt = ptp.tile([128, GW], bf16, tag="pt")
                        nc.scalar.activation(pt[:], sps[:], EXPF, scale=scale)
                        if j >= 4 * g:
                            nc.vector.tensor_mul(pt[:], pt[:], mtiles[j - 4 * g][:])
                        nc.tensor.matmul(lps[:], ones[:], pt[:],
                                         start=(j == 0), stop=(j == nj - 1))
                        nc.tensor.matmul(ops_[:], v_t[j][:], pt[:],
                                         start=(j == 0), stop=(j == nj - 1))
                    ls = nrm.tile([1, GW], f32, tag="ls")
                    nc.vector.reciprocal(ls[:], lps[:])
                    lb = nrm.tile([128, GW], f32, tag="lb")
                    nc.gpsimd.partition_broadcast(lb[:], ls[:])
                    ot = qkv.tile([128, GW], bf16, tag=f"oT{h}")
                    nc.vector.tensor_mul(ot[:], ops_[:], lb[:])
                    nc.sync.dma_start(ag_ins[g][h * 128:(h + 1) * 128, :], ot[:])

                nc.gpsimd.collective_compute(
                    "AllGather", mybir.AluOpType.bypass,
                    replica_groups=[list(range(N_CORES))],
                    ins=[ag_ins[g][:]], outs=[ag_outs[g][:]],
                )
                if g == 0:
                    for k in range(KT):
                        nc.sync.dma_start(wo_t[k][:], wod[k * 128:(k + 1) * 128, :])
                if g >= 1:
                    oproj(g - 1)
            oproj(G - 1)

    nc.compile()
    return nc


def _get_program():
    if "nc" not in _CACHE:
        _CACHE["nc"] = _build_program()
    return _CACHE["nc"]


def kernel(x, wq, wk, wv, wo):
    from concourse.bass_utils import run_bass_kernel_spmd
    from ml_dtypes import bfloat16

    nc = _get_program()

    x2 = np.asarray(x, dtype=np.float32).reshape(T, HID)
    xT = np.ascontiguousarray(x2.T).astype(bfloat16)
    cosT, sinT = _rope_tables()
    masks = _causal_masks().astype(bfloat16)
    ones = np.ones((128, 1), dtype=np.float32).astype(bfloat16)

    wq = np.asarray(wq, dtype=np.float32)
    wk = np.asarray(wk, dtype=np.float32)
    wv = np.asarray(wv, dtype=np.float32)
    wo = np.asarray(wo, dtype=np.float32)

    in_maps = []
    for i in range(N_CORES):
        in_maps.append({
            "xT": xT,
            "wq": np.ascontiguousarray(wq[:, i * DQ:(i + 1) * DQ]).astype(bfloat16),
            "wk": np.ascontiguousarray(wk[:, i * HD:(i + 1) * HD]).astype(bfloat16),
            "wv": np.ascontiguousarray(wv[:, i * HD:(i + 1) * HD]).astype(bfloat16),
            "wo": np.ascontiguousarray(wo[:, i * NO:(i + 1) * NO]).astype(bfloat16),
            "cosT": cosT,
            "sinT": sinT,
            "masks": masks,
            "ones": ones,
        })

    _CACHE["last_in_maps"] = in_maps
    res = run_bass_kernel_spmd(nc, in_maps, list(range(N_CORES)))
    _CACHE["last_result"] = res
    out = np.empty((T, HID), dtype=np.float32)
    for i in range(N_CORES):
        out[:, i * NO:(i + 1) * NO] = res.results[i]["out"]
    return out.reshape(1, T, HID)


# revision 4
# speedup vs baseline: 1.5048x; 1.1173x over previous
"""GQA attention (llama3-style RoPE, causal) on 8 trn2 NeuronCores.

Sharding: tensor-parallel over KV-head groups for QKV+attention; the o_proj
is COLUMN-parallel. Core i owns kv-head i and its 4 query heads:
wq[:, i*512:(i+1)*512], wk/wv[:, i*128:(i+1)*128], plus the COLUMN slice
wo[:, i*512:(i+1)*512]. After attention, chunked AllGathers (bf16, one per
tq-block of 512 and head-PAIR) distribute every core's head-transposed
attention output [256, 512] -> gathered [2048, 512]; each core then computes
its 512 output columns for ALL rows (out[t, i*512:(i+1)*512]) with no
further collective. The host concatenates column blocks.

Dataflow per core (everything transposed-by-design, no PE transposes):
  proj:  qT/kT [d=128, T] = w-tile.T @ xT-tile, bf16 weights/x, fp32 PSUM,
         3 PSUM banks (two half-sweeps per tq-block; x streamed twice)
  RoPE:  ACT does the half-swap, DVE the cos/sin muls; outputs bf16
  vT:    staged to DRAM bf16, read back via DMA-transpose -> v [t, d]
  sT  [tk, tq] = k-tile @ qT      (bf16)
  pT  = exp(sT/sqrt(d)) (ACT, bf16 out) * causal-mask (DVE)
  l   = ones.T @ pT               (M=1 matmul, fp32 PSUM accum over tk)
  oT  [d, tq] = v.T @ pT          (fp32 PSUM accum over tk)
  oT_norm = oT * (1/l)            (approx-recip + partition-broadcast), bf16
  oproj: out[t, 0:512] += agT-tile.T @ wo-tile  (bf16, fp32 PSUM, 32 k-tiles)
All matmuls bf16; DMA issue is split across both HWDGE queues (sync+scalar).
"""

import numpy as np

H, KV, HD, HID = 32, 8, 128, 4096
T = 2048
N_CORES = 8
QH = H // KV            # 4 query heads per core
DQ = QH * HD            # 512
KT = HID // 128         # 32 contraction tiles for projections
TN = T // 128           # 16 sequence tiles
G = 4                   # tq groups of 512
GW = T // G             # 512
NO = HID // N_CORES     # 512 output columns per core

THETA, FACTOR, HI_FF, LO_FF, ORIG_MAX = 500000.0, 8.0, 4.0, 1.0, 8192

_CACHE = {}


def _rope_tables():
    inv = 1.0 / (THETA ** (np.arange(0, HD, 2, dtype=np.float64) / HD))
    wavelen = 2.0 * np.pi / inv
    low_wl = ORIG_MAX / LO_FF
    high_wl = ORIG_MAX / HI_FF
    smooth = (ORIG_MAX / wavelen - LO_FF) / (HI_FF - LO_FF)
    scaled = np.where(wavelen > low_wl, inv / FACTOR, inv)
    mid = (wavelen <= low_wl) & (wavelen >= high_wl)
    scaled = np.where(mid, (1 - smooth) * inv / FACTOR + smooth * inv, scaled)
    inv32 = scaled.astype(np.float32)
    pos = np.arange(T, dtype=np.float32)
    freqs = pos[:, None] * inv32[None, :]          # [T, 64]
    emb = np.concatenate([freqs, freqs], axis=-1)  # [T, 128]
    cosT = np.ascontiguousarray(np.cos(emb).T)     # [128, T]
    sinT = np.ascontiguousarray(np.sin(emb).T)
    return cosT, sinT


def _causal_masks():
    # pT tile is [tk(part) 128, tq(free) 512]; within a tq-group the diagonal
    # tile sits at block v (=tk_tile - 4*g). keep where tq >= tk.
    tri = np.triu(np.ones((128, 128), dtype=np.float32))
    masks = np.zeros((4, 128, 512), dtype=np.float32)
    for v in range(4):
        for c in range(4):
            if c > v:
                masks[v, :, c * 128:(c + 1) * 128] = 1.0
            elif c == v:
                masks[v, :, c * 128:(c + 1) * 128] = tri
    return masks


def _build_program():
    import concourse.bacc as bacc
    import concourse.mybir as mybir
    from concourse.tile import TileContext

    f32 = mybir.dt.float32
    bf16 = mybir.dt.bfloat16
    EXPF = mybir.ActivationFunctionType.Exp

    nc = bacc.Bacc("TRN2", target_bir_lowering=False, debug=False,
                   num_devices=N_CORES)

    xT = nc.dram_tensor("xT", [HID, T], bf16, kind="ExternalInput")
    wqd = nc.dram_tensor("wq", [HID, DQ], bf16, kind="ExternalInput")
    wkd = nc.dram_tensor("wk", [HID, HD], bf16, kind="ExternalInput")
    wvd = nc.dram_tensor("wv", [HID, HD], bf16, kind="ExternalInput")
    wod = nc.dram_tensor("wo", [HID, NO], bf16, kind="ExternalInput")
    cosd = nc.dram_tensor("cosT", [HD, T], f32, kind="ExternalInput")
    sind = nc.dram_tensor("sinT", [HD, T], f32, kind="ExternalInput")
    maskd = nc.dram_tensor("masks", [4, HD, GW], bf16, kind="ExternalInput")
    onesd = nc.dram_tensor("ones", [128, 1], bf16, kind="ExternalInput")
    outd = nc.dram_tensor("out", [T, NO], f32, kind="ExternalOutput")

    vstage = nc.dram_tensor("vstage", [HD, T], bf16)
    # one AllGather per (tq-block, head-pair): in [256, 512] -> out [2048, 512]
    ag_ins = [[nc.dram_tensor(f"ag_in{g}_{p}", [2 * HD, GW], bf16)
               for p in range(2)] for g in range(G)]
    ag_outs = [[nc.dram_tensor(f"ag_out{g}_{p}", [2 * HD * N_CORES, GW], bf16,
                               addr_space="Shared") for p in range(2)]
               for g in range(G)]

    scale = float(1.0 / np.sqrt(HD))

    with TileContext(nc) as tc:
        with (
            tc.tile_pool(name="const", bufs=1) as cpool,
            tc.tile_pool(name="wres", bufs=1) as wres,
            tc.tile_pool(name="stream", bufs=8) as stp,
            tc.tile_pool(name="qkv", bufs=2) as qkv,
            tc.tile_pool(name="kvres", bufs=1) as kvres,
            tc.tile_pool(name="rope", bufs=2) as rtp,
            tc.tile_pool(name="pt", bufs=4) as ptp,
            tc.tile_pool(name="norm", bufs=2) as nrm,
            tc.tile_pool(name="agbuf", bufs=1) as agp,
            tc.tile_pool(name="obuf", bufs=2) as obp,
            tc.tile_pool(name="ppsum", bufs=3, space="PSUM") as pps,
            tc.tile_pool(name="spsum", bufs=2, space="PSUM") as sps_pool,
            tc.tile_pool(name="opsum", bufs=1, space="PSUM") as ops_pool,
            tc.tile_pool(name="lpsum", bufs=1, space="PSUM") as lps_pool,
            tc.tile_pool(name="ojpsum", bufs=1, space="PSUM") as ojp,
        ):
            # resident tiles, DMAs emitted lazily on first use
            cos = cpool.tile([HD, T], f32, tag="cos")
            sin = cpool.tile([HD, T], f32, tag="sin")
            ones = cpool.tile([128, 1], bf16, tag="ones")
            mtiles = [cpool.tile([HD, GW], bf16, tag=f"mask{v}", name=f"mask{v}")
                      for v in range(4)]
            wq_t = [wres.tile([128, DQ], bf16, tag=f"wq{k}", name=f"wq{k}") for k in range(KT)]
            wk_t = [wres.tile([128, HD], bf16, tag=f"wk{k}", name=f"wk{k}") for k in range(KT)]
            wv_t = [wres.tile([128, HD], bf16, tag=f"wv{k}", name=f"wv{k}") for k in range(KT)]
            wo_t = [wres.tile([128, NO], bf16, tag=f"wo{k}", name=f"wo{k}") for k in range(KT)]
            kT_t = [kvres.tile([128, GW], bf16, tag=f"kT{g}", name=f"kT{g}") for g in range(G)]
            v_t = [kvres.tile([128, 128], bf16, tag=f"v{j}", name=f"v{j}") for j in range(TN)]

            def rope_drain(dst, src, gs):
                # dst[bf16] = src*cos + rotate_half(src)*sin ; src is fp32 PSUM
                rot = rtp.tile([128, GW], f32, tag="rot")
                nc.scalar.mul(rot[0:64, :], src[64:128, :], -1.0)
                nc.scalar.copy(rot[64:128, :], src[0:64, :])
                t1 = rtp.tile([128, GW], f32, tag="t1")
                nc.vector.tensor_mul(t1[:], src[:], cos[:, gs])
                nc.vector.tensor_mul(rot[:], rot[:], sin[:, gs])
                nc.vector.tensor_add(dst, t1[:], rot[:])

            def oproj(g):
                # out[t, :NO] for t-block g from gathered attnT (all heads).
                # gathered pair-p block c holds global heads {4c+2p, 4c+2p+1};
                # pair it with wo row-tile 4c+2p+hh.
                ag_t = []     # (tile, wo_index) in accumulation order
                for p in range(2):
                    for c in range(N_CORES):
                        for hh in range(2):
                            at = agp.tile([128, GW], bf16, tag=f"ag{p}_{c}_{hh}",
                                          name=f"ag{g}_{p}_{c}_{hh}")
                            nc.scalar.dma_start(
                                at[:],
                                ag_outs[g][p][(2 * c + hh) * 128:(2 * c + hh + 1) * 128, :])
                            ag_t.append((at, 4 * c + 2 * p + hh))
                nf = len(ag_t)
                for t in range(4):
                    opj = ojp.tile([128, NO], f32, tag="oj")
                    for fi, (at, kw) in enumerate(ag_t):
                        nc.tensor.matmul(opj[:], at[:, t * 128:(t + 1) * 128],
                                         wo_t[kw][:], start=(fi == 0),
                                         stop=(fi == nf - 1))
                    ob = obp.tile([128, NO], f32, tag="ob")
                    nc.scalar.copy(ob[:], opj[:])
                    nc.scalar.dma_start(outd[g * GW + t * 128:g * GW + (t + 1) * 128, :],
                                        ob[:])

            qT_cur = [None] * QH   # per-g rotating qT tiles
            for g in range(G):
                gs = slice(g * GW, (g + 1) * GW)
                # ---- projections: two half-sweeps over 3 PSUM banks ----
                for half in range(2):
                    h0, h1 = (0, 1) if half == 0 else (2, 3)
                    pp = [pps.tile([128, GW], f32, tag="pp", name=f"pp{g}_{half}_{_i}") for _i in range(3)]
                    for k in range(KT):
                        xt = stp.tile([128, GW], bf16, tag="xt")
                        dmaq = nc.sync if k % 2 == 0 else nc.scalar
                        dmaq.dma_start(xt[:], xT[k * 128:(k + 1) * 128, gs])
                        if g == 0 and half == 0:
                            nc.sync.dma_start(wq_t[k][:], wqd[k * 128:(k + 1) * 128, :])
                            nc.scalar.dma_start(wk_t[k][:], wkd[k * 128:(k + 1) * 128, :])
                        if g == 0 and half == 1 and k == 0:
                            for kk in range(KT):
                                nc.scalar.dma_start(wv_t[kk][:],
                                                    wvd[kk * 128:(kk + 1) * 128, :])
                        st, sp = (k == 0), (k == KT - 1)
                        nc.tensor.matmul(pp[0][:], wq_t[k][:, h0 * 128:(h0 + 1) * 128],
                                         xt[:], start=st, stop=sp)
                        nc.tensor.matmul(pp[1][:], wq_t[k][:, h1 * 128:(h1 + 1) * 128],
                                         xt[:], start=st, stop=sp)
                        wkv = wk_t[k] if half == 0 else wv_t[k]
                        nc.tensor.matmul(pp[2][:], wkv[:], xt[:], start=st, stop=sp)
                    if g == 0 and half == 0:
                        nc.sync.dma_start(cos[:], cosd[:])
                        nc.sync.dma_start(sin[:], sind[:])
                        nc.sync.dma_start(ones[:], onesd[:])
                        for v in range(4):
                            nc.sync.dma_start(mtiles[v][:], maskd[v])
                    # drains (v first: it has a DRAM round-trip ahead of it)
                    if half == 1:
                        vt = qkv.tile([128, GW], bf16, tag="vT")
                        nc.scalar.copy(vt[:], pp[2][:])
                        nc.sync.dma_start(vstage[:, gs], vt[:])
                        for ts in range(4):
                            j = 4 * g + ts
                            nc.sync.dma_start_transpose(
                                v_t[j][:], vstage[:, j * 128:(j + 1) * 128])
                    for i, h in enumerate((h0, h1)):
                        qt = qkv.tile([128, GW], bf16, tag=f"qT{h}", name=f"qT{h}_{g}")
                        rope_drain(qt[:], pp[i], gs)
                        qT_cur[h] = qt
                    if half == 0:
                        rope_drain(kT_t[g][:], pp[2], gs)

                # ---- attention for tq-block g; AG fires per head-pair ----
                nj = 4 * g + 4
                for h in range(QH):
                    ops_ = ops_pool.tile([128, GW], f32, tag="op")
                    lps = lps_pool.tile([1, GW], f32, tag="lp")
                    for j in range(nj):
                        sps = sps_pool.tile([128, GW], f32, tag="sp")
                        nc.tensor.matmul(sps[:], kT_t[j // 4][:, (j % 4) * 128:(j % 4 + 1) * 128],
                                         qT_cur[h][:], start=True, stop=True)
                        pt = ptp.tile([128, GW], bf16, tag="pt")
                        nc.scalar.activation(pt[:], sps[:], EXPF, scale=scale)
                        if j >= 4 * g:
                            nc.vector.tensor_mul(pt[:], pt[:], mtiles[j - 4 * g][:])
                        nc.tensor.matmul(lps[:], ones[:], pt[:],
                                         start=(j == 0), stop=(j == nj - 1))
                        nc.tensor.matmul(ops_[:], v_t[j][:], pt[:],
                                         start=(j == 0), stop=(j == nj - 1))
                    ls = nrm.tile([1, GW], f32, tag="ls")
                    nc.vector.reciprocal_approx_fast(ls[:], lps[:])
                    lb = nrm.tile([128, GW], f32, tag="lb")
                    nc.gpsimd.partition_broadcast(lb[:], ls[:])
                    ot = qkv.tile([128, GW], bf16, tag=f"oT{h}")
                    nc.vector.tensor_mul(ot[:], ops_[:], lb[:])
                    nc.sync.dma_start(ag_ins[g][h // 2][(h % 2) * 128:(h % 2 + 1) * 128, :],
                                      ot[:])
                    if h % 2 == 1:
                        p = h // 2
                        nc.gpsimd.collective_compute(
                            "AllGather", mybir.AluOpType.bypass,
                            replica_groups=[list(range(N_CORES))],
                            ins=[ag_ins[g][p][:]], outs=[ag_outs[g][p][:]],
                        )

                if g == 0:
                    for k in range(KT):
                        nc.scalar.dma_start(wo_t[k][:], wod[k * 128:(k + 1) * 128, :])
                if g >= 1:
                    oproj(g - 1)
            oproj(G - 1)

    nc.compile()
    return nc


def _get_program():
    if "nc" not in _CACHE:
        _CACHE["nc"] = _build_program()
    return _CACHE["nc"]


def kernel(x, wq, wk, wv, wo):
    from concourse.bass_utils import run_bass_kernel_spmd
    from ml_dtypes import bfloat16

    nc = _get_program()

    x2 = np.asarray(x, dtype=np.float32).reshape(T, HID)
    xT = np.ascontiguousarray(x2.T).astype(bfloat16)
    cosT, sinT = _rope_tables()
    masks = _causal_masks().astype(bfloat16)
    ones = np.ones((128, 1), dtype=np.float32).astype(bfloat16)

    wq = np.asarray(wq, dtype=np.float32)
    wk = np.asarray(wk, dtype=np.float32)
    wv = np.asarray(wv, dtype=np.float32)
    wo = np.asarray(wo, dtype=np.float32)

    in_maps = []
    for i in range(N_CORES):
        in_maps.append({
            "xT": xT,
            "wq": np.ascontiguousarray(wq[:, i * DQ:(i + 1) * DQ]).astype(bfloat16),
            "wk": np.ascontiguousarray(wk[:, i * HD:(i + 1) * HD]).astype(bfloat16),
            "wv": np.ascontiguousarray(wv[:, i * HD:(i + 1) * HD]).astype(bfloat16),
            "wo": np.ascontiguousarray(wo[:, i * NO:(i + 1) * NO]).astype(bfloat16),
            "cosT": cosT,
            "sinT": sinT,
            "masks": masks,
            "ones": ones,
        })

    _CACHE["last_in_maps"] = in_maps
    res = run_bass_kernel_spmd(nc, in_maps, list(range(N_CORES)))
    _CACHE["last_result"] = res
    out = np.empty((T, HID), dtype=np.float32)
    for i in range(N_CORES):
        out[:, i * NO:(i + 1) * NO] = res.results[i]["out"]
    return out.reshape(1, T, HID)
